# revision 1
# baseline (speedup 1.0000x reference)
"""DGCNN segmentation kernel for Trainium2 (8 NeuronCores, data-parallel over batch).

Layout convention on device: all activations are channel-major [C, N] (channels on
partitions, points on the free dim). EdgeConv is factorized:
    edge @ W^T = (x_j - x_i) @ W1^T + x_i @ W2^T = a_j + c_i
with a = X @ W1^T, c = X @ (W2 - W1)^T, so the per-edge matmul collapses into two
point matmuls plus a gather-max of `a` over each point's 20 nearest neighbors
(leaky-relu and the positive per-channel BN scale commute with the max).
KNN keys r_ij = 2<x_i, x_j> - xx_j (row-constant -xx_i dropped; ranking-equivalent).
"""

import os
import sys
import numpy as np

sys.path.insert(0, "/opt/trn_rl_repo")

from contextlib import ExitStack

import concourse.bass as bass
import concourse.tile as tile
from concourse import bacc, mybir
from concourse import bass_utils

B = 8
N = 2048
KNN = 20
KPAD = 32  # neighbor slots padded to 32 so the 16-wrap edge-list layout is clean
NB = N // 128  # 16 row blocks
NEG = -3.0e38
F32 = mybir.dt.float32
U16 = mybir.dt.uint16
I16 = mybir.dt.int16

# (name, Cin, Cout) for the four edge convs
EDGE = [("ec1", 3, 64), ("ec2", 64, 64), ("ec3", 64, 128), ("ec4", 128, 256)]


def _ceil(a, b):
    return (a + b - 1) // b


def _kchunks(C):
    """Split a contraction dim C into partition-sized chunks."""
    out = []
    o = 0
    while o < C:
        c = min(128, C - o)
        out.append((o, c))
        o += c
    return out


def build_kernel():
    nc = bacc.Bacc("TRN2", target_bir_lowering=False, debug=False)

    # ---------------- DRAM I/O ----------------
    xyz_d = nc.dram_tensor("xyz", [N, 3], F32, kind="ExternalInput")
    ident_d = nc.dram_tensor("ident", [128, 128], F32, kind="ExternalInput")

    wdram = {}

    def win(name, shape):
        wdram[name] = nc.dram_tensor(name, list(shape), F32, kind="ExternalInput")
        return wdram[name]

    for nm, ci, co in EDGE:
        win(nm + "_wt", (len(_kchunks(ci)), 128, 2 * co))
        win(nm + "_s", (_ceil(co, 128), 128, 1))
        win(nm + "_t", (_ceil(co, 128), 128, 1))
    win("fuse_wt", (5, 128, 512))   # k-chunks 64,64,128,128,128
    win("fuse_s", (4, 128, 1))
    win("fuse_t", (4, 128, 1))
    win("emb_wt", (4, 128, 1024))
    win("emb_s", (8, 128, 1))
    win("emb_t", (8, 128, 1))
    win("h1a_wt", (4, 128, 256))   # h1 cols 0..511   (applies to x_local)
    win("h1b_wt", (8, 128, 256))   # h1 cols 512..1535 (applies to x_glob)
    win("h1_s", (2, 128, 1))
    win("h1_t", (2, 128, 1))
    win("h2_wt", (2, 128, 256))
    win("h2_s", (2, 128, 1))
    win("h2_t", (2, 128, 1))
    win("h3_wt", (2, 128, 13))
    win("h3_b", (13, 1))

    out_d = nc.dram_tensor("logits", [N, 13], F32, kind="ExternalOutput")
    elist_d = nc.dram_tensor("elist_scratch", [16, N * KPAD // 16], U16)

    def load_w(pool, names):
        out = {}
        for name in names:
            t = wdram[name]
            sh = list(t.shape)
            if len(sh) == 3:
                tiles = []
                for ki in range(sh[0]):
                    w = pool.tile([sh[1], sh[2]], F32, tag=f"w_{name}_{ki}",
                                  name=f"w_{name}_{ki}")
                    nc.sync.dma_start(w[:], t[ki])
                    tiles.append(w)
                out[name] = tiles
            else:
                w = pool.tile(sh, F32, tag=f"w_{name}", name=f"w_{name}")
                nc.sync.dma_start(w[:], t[:])
                out[name] = w
        return out

    with tile.TileContext(nc) as tc, ExitStack() as ctx:
        from concourse import library_config
        nc.gpsimd.load_library(library_config.ap_gather)

        const_pool = ctx.enter_context(tc.tile_pool(name="consts", bufs=1))
        feat_pool = ctx.enter_context(tc.tile_pool(name="feat", bufs=1))

        ident = const_pool.tile([128, 128], F32, tag="ident")
        nc.sync.dma_start(ident[:], ident_d[:])
        ones_col = const_pool.tile([128, 1], F32, tag="ones_col")
        nc.vector.memset(ones_col[:], 1.0)
        ones_r = const_pool.tile([1, 128], F32, tag="ones_r")
        nc.vector.memset(ones_r[:], 1.0)

        # xyz -> channel-major; L carries a trailing ones row (matmul lhsT side),
        # R carries a trailing -xx/2 row (matmul rhs side) so the distance keys
        # come out of a single PE accumulation group.
        # aux rows must start at a partition multiple of 32: xyz sits in rows
        # 0-2, rows 3-31 are zeroed, the ones/-xx row lives at partition 32
        x0L = feat_pool.tile([33, N], F32, tag="x0L")
        x0R = feat_pool.tile([33, N], F32, tag="x0R")
        nc.vector.memset(x0L[0:32, :], 0.0)
        nc.vector.memset(x0R[0:32, :], 0.0)
        nc.sync.dma_start(x0L[0:3, :], xyz_d.ap().rearrange("n c -> c n"))
        nc.sync.dma_start(x0R[0:3, :], xyz_d.ap().rearrange("n c -> c n"))
        nc.vector.memset(x0L[32:33, :], 1.0)

        x1L = feat_pool.tile([65, N], F32, tag="x1L")
        x1R = feat_pool.tile([65, N], F32, tag="x1R")
        nc.vector.memset(x1L[64:65, :], 1.0)
        x2t = feat_pool.tile([64, N], F32, tag="x2t")
        x3t = feat_pool.tile([128, N], F32, tag="x3t")
        x4a = feat_pool.tile([128, N], F32, tag="x4a")
        x4b = feat_pool.tile([128, N], F32, tag="x4b")

        def lrelu_affine(dst, src, s_ap, t_ap):
            # y = s*src + t; lrelu(y) = max(0.2*y, y)
            nc.scalar.activation(dst, src, mybir.ActivationFunctionType.Identity,
                                 bias=t_ap, scale=s_ap)
            nc.vector.scalar_tensor_tensor(dst, dst, 0.2, dst,
                                           op0=mybir.AluOpType.mult,
                                           op1=mybir.AluOpType.max)

        # ================= edge conv phase =================
        with ExitStack() as ectx:
            ewpool = ectx.enter_context(tc.tile_pool(name="eweights", bufs=1))
            dpool = ectx.enter_context(tc.tile_pool(name="dtile", bufs=3))
            dpsum_pool = ectx.enter_context(tc.tile_pool(name="dpsum", bufs=2, space="PSUM"))
            fpsum_pool = ectx.enter_context(tc.tile_pool(name="fpsum", bufs=3, space="PSUM"))
            topk_pool = ectx.enter_context(tc.tile_pool(name="topk", bufs=3))
            gath_pool = ectx.enter_context(tc.tile_pool(name="gath", bufs=2))
            acat_pool = ectx.enter_context(tc.tile_pool(name="acat", bufs=2))
            mt_pool = ectx.enter_context(tc.tile_pool(name="mt", bufs=2))
            escr_pool = ectx.enter_context(tc.tile_pool(name="escr", bufs=2))
            nxx_pool = ectx.enter_context(tc.tile_pool(name="nxx", bufs=1))
            elist_pool = ectx.enter_context(tc.tile_pool(name="elist", bufs=3))

            ew = load_w(ewpool, [nm + sfx for nm, ci, co in EDGE
                                 for sfx in ("_wt", "_s", "_t")])

            # per-layer input feature APs (channel-chunked) and output APs.
            # fold=(lhsT_D, rhs_D, negxx_dst) when the -xx row rides inside the
            # single distance matmul (C+1 <= 128); else None -> ones_r matmul.
            layer_in = [
                [x0L[0:3, :]],
                [x1L[0:64, :]],
                [x2t[:, :]],
                [x3t[:, :]],
            ]
            layer_out = [
                [x1L[0:64, :]],
                [x2t[:, :]],
                [x3t[:, :]],
                [x4a[:, :], x4b[:, :]],
            ]
            layer_fold = [
                ([x0L[0:33, :]], [x0R[0:33, :]], x0R[32:33, :]),
                ([x1L[0:65, :]], [x1R[0:65, :]], x1R[64:65, :]),
                None,
                None,
            ]

            for li, (nm, ci, co) in enumerate(EDGE):
                feats = layer_in[li]
                kchunks = _kchunks(ci)
                assert len(kchunks) == len(feats)

                # ---- negxxh row [1, N] = -xx/2 ----
                fold = layer_fold[li]
                if fold is None:
                    negxxh = nxx_pool.tile([1, N], F32, tag="negxxh", name=f"negxxh_{li}")
                else:
                    negxxh = fold[2]
                for nci in range(4):
                    ns = slice(nci * 512, (nci + 1) * 512)
                    xxp = fpsum_pool.tile([1, 512], F32, tag="fpsum", name=f"xxp_{li}_{nci}")
                    for ki, (ko, kc) in enumerate(kchunks):
                        x2s = escr_pool.tile([128, 512], F32, tag="x2s", name=f"x2s_{li}_{nci}_{ki}")
                        nc.scalar.square(x2s[:kc, :], feats[ki][:, ns])
                        nc.tensor.matmul(xxp[:], ones_col[:kc, :], x2s[:kc, :],
                                         start=(ki == 0), stop=(ki == len(kchunks) - 1))
                    nc.scalar.mul(negxxh[:, ns], xxp[:], -0.5)

                # ---- a-part matmul: a = Wg @ x  -> atiles [co rows] ----
                wts = ew[nm + "_wt"]
                n_at = _ceil(co, 128)
                atiles = [acat_pool.tile([128, N], F32, tag=f"ac{ai}", name=f"a_{li}_{ai}")
                          for ai in range(n_at)]
                for ai in range(n_at):
                    mw = min(128, co - ai * 128)
                    for nci in range(4):
                        ns = slice(nci * 512, (nci + 1) * 512)
                        fp = fpsum_pool.tile([128, 512], F32, tag="fpsum", name=f"afp_{li}_{ai}_{nci}")
                        for ki, (ko, kc) in enumerate(kchunks):
                            nc.tensor.matmul(
                                fp[:mw, :],
                                wts[ki][:kc, ai * 128:ai * 128 + mw],
                                feats[ki][:, ns],
                                start=(ki == 0), stop=(ki == len(kchunks) - 1))
                        nc.scalar.copy(atiles[ai][:mw, ns], fp[:mw, :])

                # ---- KNN + gather + k-max, pipelined per 128-row block ----
                n_ct = _ceil(co, 128)
                mouts = [mt_pool.tile([128, N], F32, tag=f"mt{ai}", name=f"m_{li}_{ai}")
                         for ai in range(n_ct)]

                # reduces are deferred by one block so the DVE never stalls on
                # an in-flight gather (DVE executes in program order)
                pending = []

                def flush_reduces():
                    while pending:
                        gt_, ai_, ms_, ch_ = pending.pop(0)
                        if not os.environ.get("ABL_NOREDUCE"):
                            nc.vector.tensor_reduce(
                                mouts[ai_][:ch_, ms_],
                                gt_[:ch_, :].rearrange("c (p k) -> c p k", k=KPAD)[:, :, 0:KNN],
                                axis=mybir.AxisListType.X, op=mybir.AluOpType.max)

                for b in range(NB):
                    ms = slice(b * 128, (b + 1) * 128)
                    for half in range(2):
                        dps = dpsum_pool.tile([128, N // 2], F32, tag="dpsum",
                                              name=f"dps_{li}_{b}_{half}")
                        for nci in range(2):
                            ns = slice(half * 1024 + nci * 512,
                                       half * 1024 + (nci + 1) * 512)
                            nsp = slice(nci * 512, (nci + 1) * 512)
                            if fold is not None:
                                lhsT_D, rhs_D, _ = fold
                                nc.tensor.matmul(dps[:, nsp], lhsT_D[0][:, ms],
                                                 rhs_D[0][:, ns],
                                                 start=True, stop=True)
                            else:
                                for ki, (ko, kc) in enumerate(kchunks):
                                    nc.tensor.matmul(dps[:, nsp], feats[ki][:, ms],
                                                     feats[ki][:, ns],
                                                     start=(ki == 0), stop=False)
                                nc.tensor.matmul(dps[:, nsp], ones_r[:, 0:128],
                                                 negxxh[:, ns], start=False, stop=True)
                        if half == 0:
                            dt = dpool.tile([128, N], F32, tag="dtile",
                                            name=f"dt_{li}_{b}")
                        nc.scalar.copy(dt[:, half * 1024:(half + 1) * 1024], dps[:])

                    # level 1: per-128-col-chunk top-8 (prob. exact: a chunk
                    # holding >8 of the true top-20 is ~1e-5 per row)
                    cand = topk_pool.tile([128, 128], F32, tag="cand", name=f"cand_{li}_{b}")
                    for c in range(16 if not os.environ.get("ABL_NOTOPK") else 0):
                        nc.vector.max(cand[:, c * 8:(c + 1) * 8], dt[:, c * 128:(c + 1) * 128])
                    # level 2: exact top-24 of the 128 candidates; index recovery
                    # against the unmodified full row
                    v8 = topk_pool.tile([128, 8], F32, tag="v8", name=f"v8_{li}_{b}")
                    idx32 = topk_pool.tile([128, KPAD], U16, tag="idx32", name=f"idx32_{li}_{b}")
                    nc.vector.memset(idx32[:], 0)
                    nrounds = 3 if not os.environ.get("ABL_NOTOPK") else 0
                    for r in range(nrounds):
                        nc.vector.max(v8[:], cand[:])
                        if r == 0:
                            flush_reduces()  # previous block's k-max, gather done by now
                        nc.vector.max_index(idx32[:, r * 8:(r + 1) * 8], v8[:], dt[:])
                        if r < 2:
                            nc.vector.match_replace(cand[:], v8[:], cand[:], NEG)
                    if nrounds == 0:
                        flush_reduces()

                    dst = elist_d.ap().rearrange("tl (p th) -> p tl th", th=2)
                    for th in range(2):
                        nc.sync.dma_start(dst[b * 128:(b + 1) * 128, :, th],
                                          idx32[:, th * 16:(th + 1) * 16])

                    # per-block edge-list segment, replicated to all 8 core groups
                    ngroups = _ceil(min(128, co), 16)
                    eseg = elist_pool.tile([128, 256], U16, tag="eseg",
                                           name=f"eseg_{li}_{b}")
                    for g in range(ngroups):
                        nc.scalar.dma_start(eseg[g * 16:(g + 1) * 16, :],
                                            elist_d[:, b * 256:(b + 1) * 256])

                    for ai in range(n_ct):
                        if os.environ.get("ABL_NOGATHER"):
                            continue
                        ch = min(128, co - ai * 128)
                        ch16 = _ceil(ch, 16) * 16
                        gt = gath_pool.tile([128, 128 * KPAD], F32, tag="gath",
                                            name=f"g_{li}_{ai}_{b}")
                        nc.gpsimd.ap_gather(
                            gt[:ch16, :], atiles[ai][:ch16, :],
                            eseg[:ch16, :].bitcast(I16),
                            channels=ch16, num_elems=N, d=1, num_idxs=128 * KPAD)
                        pending.append((gt, ai, ms, ch))

                flush_reduces()

                # ---- c-part matmul into freed acat slots; u = M + c; lrelu ----
                for ai in range(n_ct):
                    ch = min(128, co - ai * 128)
                    ctile = acat_pool.tile([128, N], F32, tag=f"ac{ai}", name=f"c_{li}_{ai}")
                    for nci in range(4):
                        ns = slice(nci * 512, (nci + 1) * 512)
                        fp = fpsum_pool.tile([128, 512], F32, tag="fpsum", name=f"cfp_{li}_{ai}_{nci}")
                        for ki, (ko, kc) in enumerate(kchunks):
                            nc.tensor.matmul(
                                fp[:ch, :],
                                wts[ki][:kc, co + ai * 128:co + ai * 128 + ch],
                                feats[ki][:, ns],
                                start=(ki == 0), stop=(ki == len(kchunks) - 1))
                        nc.scalar.copy(ctile[:ch, ns], fp[:ch, :])
                    nc.vector.tensor_add(mouts[ai][:ch, :], mouts[ai][:ch, :], ctile[:ch, :])
                    lrelu_affine(layer_out[li][ai][:, :], mouts[ai][:ch, :],
                                 ew[nm + "_s"][ai][:ch], ew[nm + "_t"][ai][:ch])
                    if li == 0:
                        nc.scalar.copy(x1R[0:64, :], x1L[0:64, :])

        # ================= MLP head phase =================
        with ExitStack() as mctx:
            mwpool = mctx.enter_context(tc.tile_pool(name="mweights", bufs=1))
            mlp_pool = mctx.enter_context(tc.tile_pool(name="mlp", bufs=1))
            mscr_pool = mctx.enter_context(tc.tile_pool(name="mscr", bufs=2))
            mpsum_pool = mctx.enter_context(tc.tile_pool(name="mpsum", bufs=4, space="PSUM"))

            mw = load_w(mwpool, ["fuse_wt", "fuse_s", "fuse_t", "emb_wt", "emb_s",
                                 "emb_t", "h1a_wt", "h1b_wt", "h1_s", "h1_t",
                                 "h2_wt", "h2_s", "h2_t", "h3_wt", "h3_b"])

            # x_cat k-chunks for fuse: 64,64,128,128,128
            xcat_chunks = [x1L[0:64, :], x2t[:, :], x3t[:, :], x4a[:, :], x4b[:, :]]
            xcat_kc = [64, 64, 128, 128, 128]

            xl = [mlp_pool.tile([128, N], F32, tag=f"xl_{i}", name=f"xl_{i}")
                  for i in range(4)]
            for mi in range(4):
                for nci in range(4):
                    ns = slice(nci * 512, (nci + 1) * 512)
                    fp = mpsum_pool.tile([128, 512], F32, tag="mpsum", name=f"fufp_{mi}_{nci}")
                    for ki in range(5):
                        kc = xcat_kc[ki]
                        nc.tensor.matmul(fp[:], mw["fuse_wt"][ki][:kc, mi * 128:(mi + 1) * 128],
                                         xcat_chunks[ki][:, ns], start=(ki == 0), stop=(ki == 4))
                    lrelu_affine(xl[mi][:, ns], fp[:],
                                 mw["fuse_s"][mi][:], mw["fuse_t"][mi][:])

            # emb + global max -> xg [1024 rows as 8 tiles of [128, 1]]
            # x_glob: lrelu and the positive BN scale commute with the max, so
            # reduce the raw matmul output straight from PSUM and apply the
            # activation to the single reduced column.
            xg = [mlp_pool.tile([128, 1], F32, tag=f"xg_{i}", name=f"xg_{i}")
                  for i in range(8)]
            for mi in range(8):
                gmax = mscr_pool.tile([128, 4], F32, tag="gmax", name=f"gmax_{mi}")
                for nci in range(4):
                    ns = slice(nci * 512, (nci + 1) * 512)
                    fp = mpsum_pool.tile([128, 512], F32, tag="mpsum", name=f"emfp_{mi}_{nci}")
                    for ki in range(4):
                        nc.tensor.matmul(fp[:], mw["emb_wt"][ki][:, mi * 128:(mi + 1) * 128],
                                         xl[ki][:, ns], start=(ki == 0), stop=(ki == 3))
                    nc.vector.tensor_reduce(gmax[:, nci:nci + 1], fp[:],
                                            axis=mybir.AxisListType.X, op=mybir.AluOpType.max)
                raw = mscr_pool.tile([128, 1], F32, tag="xgraw", name=f"xgraw_{mi}")
                nc.vector.tensor_reduce(raw[:], gmax[:],
                                        axis=mybir.AxisListType.X, op=mybir.AluOpType.max)
                lrelu_affine(xg[mi][:], raw[:], mw["emb_s"][mi][:], mw["emb_t"][mi][:])

            # h1 global part + combined bias
            h1g = mscr_pool.tile([128, 2], F32, tag="h1g")
            for mi in range(2):
                gp = mpsum_pool.tile([128, 1], F32, tag="mpsum", name=f"gp_{mi}")
                for ki in range(8):
                    nc.tensor.matmul(gp[:], mw["h1b_wt"][ki][:, mi * 128:(mi + 1) * 128],
                                     xg[ki][:], start=(ki == 0), stop=(ki == 7))
                nc.vector.tensor_copy(h1g[:, mi:mi + 1], gp[:])
            b1p = mscr_pool.tile([128, 2], F32, tag="b1p")
            for mi in range(2):
                nc.scalar.activation(b1p[:, mi:mi + 1], h1g[:, mi:mi + 1],
                                     mybir.ActivationFunctionType.Identity,
                                     bias=mw["h1_t"][mi][:], scale=mw["h1_s"][mi][:])

            # h1 -> h2 -> h3 -> logits per n-chunk
            for nci in range(4):
                ns = slice(nci * 512, (nci + 1) * 512)
                h1t = [mscr_pool.tile([128, 512], F32, tag=f"h1_{mi}", name=f"h1_{nci}_{mi}")
                       for mi in range(2)]
                for mi in range(2):
                    fp = mpsum_pool.tile([128, 512], F32, tag="mpsum", name=f"h1fp_{nci}_{mi}")
                    for ki in range(4):
                        nc.tensor.matmul(fp[:], mw["h1a_wt"][ki][:, mi * 128:(mi + 1) * 128],
                                         xl[ki][:, ns], start=(ki == 0), stop=(ki == 3))
                    lrelu_affine(h1t[mi][:], fp[:], mw["h1_s"][mi][:], b1p[:, mi:mi + 1])
                h2t = [mscr_pool.tile([128, 512], F32, tag=f"h2_{mi}", name=f"h2_{nci}_{mi}")
                       for mi in range(2)]
                for mi in range(2):
                    fp = mpsum_pool.tile([128, 512], F32, tag="mpsum", name=f"h2fp_{nci}_{mi}")
                    for ki in range(2):
                        nc.tensor.matmul(fp[:], mw["h2_wt"][ki][:, mi * 128:(mi + 1) * 128],
                                         h1t[ki][:], start=(ki == 0), stop=(ki == 1))
                    lrelu_affine(h2t[mi][:], fp[:], mw["h2_s"][mi][:], mw["h2_t"][mi][:])
                lt = mscr_pool.tile([13, 512], F32, tag="logitsT", name=f"lt_{nci}")
                fp = mpsum_pool.tile([13, 512], F32, tag="mpsum", name=f"h3fp_{nci}")
                for ki in range(2):
                    nc.tensor.matmul(fp[:], mw["h3_wt"][ki][:, 0:13], h2t[ki][:],
                                     start=(ki == 0), stop=(ki == 1))
                nc.scalar.activation(lt[:], fp[:], mybir.ActivationFunctionType.Identity,
                                     bias=mw["h3_b"][:])
                for tb in range(4):
                    tp = mpsum_pool.tile([128, 13], F32, tag="mpsum", name=f"tp_{nci}_{tb}")
                    nc.tensor.transpose(tp[:], lt[:, tb * 128:(tb + 1) * 128], ident[0:13, 0:13])
                    ot = mscr_pool.tile([128, 13], F32, tag="otile", name=f"ot_{nci}_{tb}")
                    nc.scalar.copy(ot[:], tp[:])
                    nc.sync.dma_start(
                        out_d[nci * 512 + tb * 128: nci * 512 + (tb + 1) * 128, :], ot[:])

    nc.compile()
    return nc


def _prep_weights(inputs):
    """Host-side folding of weights into the shapes the device kernel expects."""
    f = {}

    def chunked_vec(v):
        v = np.asarray(v, np.float32).reshape(-1)
        nk = _ceil(len(v), 128)
        out = np.zeros((nk, 128, 1), np.float32)
        for ki in range(nk):
            c = min(128, len(v) - ki * 128)
            out[ki, :c, 0] = v[ki * 128:ki * 128 + c]
        return out

    def kchunked_T(w):
        # w: [O, C] -> transposed [C, O] -> padded k-chunks [nk, 128, O]
        wt = np.ascontiguousarray(w.T.astype(np.float32))
        C, O = wt.shape
        nk = _ceil(C, 128)
        out = np.zeros((nk, 128, O), np.float32)
        for ki in range(nk):
            c = min(128, C - ki * 128)
            out[ki, :c, :] = wt[ki * 128:ki * 128 + c]
        return out

    for nm, ci, co in EDGE:
        w = np.asarray(inputs[nm + "_w"], np.float32)  # [O, 2C]
        w1 = w[:, :ci]
        w2 = w[:, ci:]
        wg = w1                 # applies to x_j (gathered)
        wc = w2 - w1            # applies to x_i (center)
        wcat = np.concatenate([wg, wc], axis=0)  # [2O, C]
        f[nm + "_wt"] = kchunked_T(wcat)
        f[nm + "_s"] = chunked_vec(inputs[nm + "_s"])
        f[nm + "_t"] = chunked_vec(inputs[nm + "_t"])

    fw = np.ascontiguousarray(np.asarray(inputs["fuse_w"], np.float32).T)  # [512, 512]
    fko = [0, 64, 128, 256, 384, 512]
    fwt = np.zeros((5, 128, 512), np.float32)
    for ki in range(5):
        kc = fko[ki + 1] - fko[ki]
        fwt[ki, :kc, :] = fw[fko[ki]:fko[ki + 1]]
    f["fuse_wt"] = fwt
    f["fuse_s"] = chunked_vec(inputs["fuse_s"])
    f["fuse_t"] = chunked_vec(inputs["fuse_t"])
    f["emb_wt"] = kchunked_T(np.asarray(inputs["emb_w"], np.float32))
    f["emb_s"] = chunked_vec(inputs["emb_s"])
    f["emb_t"] = chunked_vec(inputs["emb_t"])
    h1w = np.asarray(inputs["h1_w"], np.float32)  # [256, 1536]
    f["h1a_wt"] = kchunked_T(h1w[:, :512])
    f["h1b_wt"] = kchunked_T(h1w[:, 512:])
    f["h1_s"] = chunked_vec(inputs["h1_s"])
    f["h1_t"] = chunked_vec(inputs["h1_t"])
    f["h2_wt"] = kchunked_T(np.asarray(inputs["h2_w"], np.float32))
    f["h2_s"] = chunked_vec(inputs["h2_s"])
    f["h2_t"] = chunked_vec(inputs["h2_t"])
    f["h3_wt"] = kchunked_T(np.asarray(inputs["h3_w"], np.float32))
    f["h3_b"] = np.asarray(inputs["h3_b"], np.float32).reshape(-1, 1)
    f["ident"] = np.eye(128, dtype=np.float32)
    return f


_NC_CACHE = {}


def _get_nc():
    if "nc" not in _NC_CACHE:
        _NC_CACHE["nc"] = build_kernel()
    return _NC_CACHE["nc"]


def kernel(**inputs):
    xyz = np.asarray(inputs["xyz"], np.float32)
    assert xyz.shape == (B, N, 3)
    w = _prep_weights(inputs)
    nc = _get_nc()
    in_maps = []
    for b in range(B):
        m = {"xyz": np.ascontiguousarray(xyz[b])}
        m.update(w)
        in_maps.append(m)
    res = bass_utils.run_bass_kernel_spmd(nc, in_maps, core_ids=list(range(B)))
    out = np.stack([res.results[b]["logits"] for b in range(B)], axis=0)
    return out.astype(np.float32)

